# revision 1
# baseline (speedup 1.0000x reference)
"""Cross-attention Trainium2 kernel (nn_CrossAttention_24575802868332).

Sharding: 8 cores; core c handles batch b = c//4 and query rows
r = (c%4)*1024 .. +1024.  Embarrassingly parallel, no collectives.
Host pre-transposes x and context slices (pure layout work).

Per-core on-device computation (all big matmuls fp32r, 1 cyc/row):
  P1: q^T = Wq^T @ x^T                       qt_sb [128, 4, 1024]
  P2: per m-block of 512 (8 blocks):
      k^T = Wk^T @ ctx^T block               kt [128, 4, 512]
      v   = ctx^T.T @ Wv block, + ones col   vt [128, 4, 8, 65]
      per (head-pair, q-chunk, m-subtile):
        S^T pair (row-packed K=64 matmuls) -> psum slab [128, 1024]
        P = exp(0.125 * S^T) on ScalarE    -> sbuf fp32r slab
        O_aug[65,512] += v_aug.T @ P        (row 64 = softmax denom l)
      flush O_aug psums -> acc_o [65, 16, 512] (DVE copy/add)
  P3: l -> partition 0 via SBUF DMA, reciprocal, K=1 ones outer-product
      broadcast, normalize acc_o in place (out viewed fp32r),
      repack head pairs into ko_sb [128, 8, 512] via SBUF->SBUF DMA
  P4: out = (O/l) @ Wo + bo (bias via K=1 ones matmul), DMA out
"""

import os
import sys

sys.path.insert(0, "/opt/trn_rl_repo")

from contextlib import ExitStack

import numpy as np

import concourse.bass as bass
import concourse.tile as tile
from concourse import bacc, mybir

F32 = mybir.dt.float32
F32R = mybir.dt.float32r
AF = mybir.ActivationFunctionType

# Problem constants (hardcoded per contract)
B, N, M = 2, 4096, 4096
DQ, DC, INNER = 1024, 768, 512
H, D = 8, 64
NCORES = 8
NQ = N * B // NCORES  # 1024 query rows per core
QC = 2  # q chunks of 512
MBLK = 512  # m block size
NBLK = M // MBLK  # 8
HP = H // 2  # 4 head pairs
KQ = DQ // 128  # 8 k-chunks for q proj
KC = DC // 128  # 6 k-chunks for k/v proj
MS = MBLK // 128  # 4 m-subtiles per block


def build_nc():
    nc = bacc.Bacc(
        "TRN2",
        target_bir_lowering=False,
        debug=False,
        enable_asserts=False,
        num_devices=NCORES,
    )
    xT = nc.dram_tensor("xT", [DQ, NQ], F32R, kind="ExternalInput").ap()
    ctxT = nc.dram_tensor("ctxT", [DC, M], F32R, kind="ExternalInput").ap()
    wq = nc.dram_tensor("wq", [DQ, INNER], F32R, kind="ExternalInput").ap()
    wk = nc.dram_tensor("wk", [DC, INNER], F32R, kind="ExternalInput").ap()
    wv = nc.dram_tensor("wv", [DC, INNER], F32R, kind="ExternalInput").ap()
    wo = nc.dram_tensor("wo", [INNER, DQ], F32R, kind="ExternalInput").ap()
    bo = nc.dram_tensor("bo", [1, DQ], F32R, kind="ExternalInput").ap()
    ones_d = nc.dram_tensor("ones_d", [1, 128], F32R, kind="ExternalInput").ap()
    out = nc.dram_tensor("out", [NQ, DQ], F32, kind="ExternalOutput").ap()

    with tile.TileContext(nc) as tc:
        _emit(nc, tc, xT, ctxT, wq, wk, wv, wo, bo, ones_d, out)
    nc.compile()
    return nc


def _emit(nc, tc, xT, ctxT, wq, wk, wv, wo, bo, ones_d, out):
    with ExitStack() as ctx:
        consts = ctx.enter_context(tc.tile_pool(name="consts", bufs=1))
        # ---- constants ----
        wq_sb = consts.tile([128, KQ, INNER], F32R, tag="wq")
        nc.sync.dma_start(out=wq_sb, in_=wq.rearrange("(k p) n -> p k n", p=128))
        wk_sb = consts.tile([128, KC, INNER], F32R, tag="wk")
        nc.sync.dma_start(out=wk_sb, in_=wk.rearrange("(k p) n -> p k n", p=128))
        wv_sb = consts.tile([128, KC, INNER], F32R, tag="wv")
        nc.sync.dma_start(out=wv_sb, in_=wv.rearrange("(k p) n -> p k n", p=128))
        wo_sb = consts.tile([128, INNER // 128, DQ], F32R, tag="wo")
        nc.sync.dma_start(out=wo_sb, in_=wo.rearrange("(k p) n -> p k n", p=128))
        bo_sb = consts.tile([1, DQ], F32R, tag="bo")
        nc.sync.dma_start(out=bo_sb, in_=bo)
        ones_row = consts.tile([1, 128], F32R, tag="ones_row")
        nc.sync.dma_start(out=ones_row, in_=ones_d)
        # ones replicated to all 128 partitions (v_aug ones column source)
        ones_col = consts.tile([128, MS * H], F32R, tag="ones_col")
        ones_bcast = bass.AP(
            tensor=ones_d.tensor, offset=0, ap=[[0, 128], [1, MS * H]]
        )
        nc.gpsimd.dma_start(out=ones_col, in_=ones_bcast)

        # persistent accumulators
        acc = ctx.enter_context(tc.tile_pool(name="acc", bufs=1))
        # acc_o[d(0:64)+l(64), slot j = hp*4 + parity*2 + qc, q 512]
        acc_o = acc.tile([65, 16, 512], F32, tag="acc_o")
        qt_sb = acc.tile([128, HP, NQ], F32R, tag="qt")  # q^T [inner, q]

        # ---- P1: q^T projection (kc-outer, 8 psum banks) ----
        with tc.tile_pool(name="xt", bufs=2) as xt_pool, tc.tile_pool(
            name="qps", bufs=8, space="PSUM"
        ) as qps:
            qt_ps = [
                [
                    qps.tile([128, 512], F32, tag="qps", name=f"qtps_{it}_{qc}")
                    for qc in range(QC)
                ]
                for it in range(HP)
            ]
            xTr = xT.rearrange("(k p) q -> p k q", p=128)
            for kc in range(KQ):
                xt = xt_pool.tile([128, NQ], F32R, tag="xt")
                nc.sync.dma_start(out=xt, in_=xTr[:, kc, :])
                for it in range(HP):
                    for qc in range(QC):
                        nc.tensor.matmul(
                            qt_ps[it][qc],
                            wq_sb[:, kc, it * 128 : (it + 1) * 128],
                            xt[:, qc * 512 : (qc + 1) * 512],
                            start=(kc == 0),
                            stop=(kc == KQ - 1),
                        )
            for it in range(HP):
                for qc in range(QC):
                    nc.vector.tensor_copy(
                        qt_sb[:, it, qc * 512 : (qc + 1) * 512], qt_ps[it][qc]
                    )

        # ---- P2: m-block loop ----
        with ExitStack() as actx:
            ctx_pool = actx.enter_context(tc.tile_pool(name="ctx", bufs=2))
            kt_pool = actx.enter_context(tc.tile_pool(name="kt", bufs=2))
            v_pool = actx.enter_context(tc.tile_pool(name="v", bufs=2))
            p_pool = actx.enter_context(tc.tile_pool(name="p", bufs=3))
            s_ps = actx.enter_context(
                tc.tile_pool(name="sps", bufs=2, space="PSUM")
            )
            o_ps = actx.enter_context(
                tc.tile_pool(name="ops", bufs=4, space="PSUM")
            )
            ctxTr = ctxT.rearrange("(k p) m -> p k m", p=128)

            def make_block_thunks(blk):
                """Per-block projection work as single-instruction thunks,
                for sprinkling among the previous block's attention slabs."""
                st = {}
                th = []

                def t_dma(blk=blk):
                    cx = ctx_pool.tile(
                        [128, KC, MBLK], F32R, tag="cx", name=f"cx{blk}"
                    )
                    m0 = blk * MBLK
                    nc.sync.dma_start(out=cx, in_=ctxTr[:, :, m0 : m0 + MBLK])
                    st["cx"] = cx
                    st["kt"] = kt_pool.tile(
                        [128, HP, MBLK], F32R, tag="kt", name=f"kt{blk}"
                    )
                    vt = v_pool.tile(
                        [128, MS, H, 65], F32R, tag="vt", name=f"vt{blk}"
                    )
                    st["vt"] = vt
                    nc.vector.tensor_copy(
                        vt[:, :, :, 64:65],
                        ones_col[:].rearrange("p (a h o) -> p a h o", a=MS, h=H),
                    )

                th.append(t_dma)
                for it in range(HP):
                    for kc in range(KC):
                        def t_kmm(it=it, kc=kc, blk=blk):
                            if kc == 0:
                                st[f"kp{it}"] = o_ps.tile(
                                    [128, 512], F32, tag="ops",
                                    name=f"kp{blk}_{it}",
                                )
                            nc.tensor.matmul(
                                st[f"kp{it}"],
                                wk_sb[:, kc, it * 128 : (it + 1) * 128],
                                st["cx"][:, kc, :],
                                start=(kc == 0),
                                stop=(kc == KC - 1),
                            )
                        th.append(t_kmm)

                    def t_kev(it=it):
                        nc.vector.tensor_copy(st["kt"][:, it, :], st[f"kp{it}"])

                    th.append(t_kev)
                for ms in range(MS):
                    for kc in range(KC):
                        def t_vmm(ms=ms, kc=kc, blk=blk):
                            if kc == 0:
                                st[f"vp{ms}"] = o_ps.tile(
                                    [128, 512], F32, tag="ops",
                                    name=f"vp{blk}_{ms}",
                                )
                            nc.tensor.matmul(
                                st[f"vp{ms}"],
                                st["cx"][:, kc, ms * 128 : (ms + 1) * 128],
                                wv_sb[:, kc, :],
                                start=(kc == 0),
                                stop=(kc == KC - 1),
                            )
                        th.append(t_vmm)

                    def t_vev(ms=ms):
                        nc.vector.tensor_copy(
                            st["vt"][:, ms, :, 0:64],
                            st[f"vp{ms}"][:].rearrange("p (h d) -> p h d", h=H),
                        )

                    th.append(t_vev)
                return st, th

            # prologue: project block 0 eagerly
            cur_st, th0 = make_block_thunks(0)
            for t in th0:
                t()

            for blk in range(NBLK):
                kt = cur_st["kt"]
                vt = cur_st["vt"]
                if blk + 1 < NBLK:
                    next_st, pend = make_block_thunks(blk + 1)
                else:
                    next_st, pend = None, []
                # pop ~evenly over the 32 slab iterations
                n_slabs = HP * QC * MS
                per = (len(pend) + n_slabs - 1) // n_slabs if pend else 0

                slab_i = 0
                for hp in range(HP):
                    for qc in range(QC):
                        ops_e = o_ps.tile(
                            [65, 512], F32, tag="ops", name=f"oe{blk}_{hp}_{qc}"
                        )
                        ops_o = o_ps.tile(
                            [65, 512], F32, tag="ops", name=f"oo{blk}_{hp}_{qc}"
                        )
                        o_emits = []
                        for mt in range(MS):
                            sl = s_ps.tile(
                                [128, 1024], F32, tag="sps",
                                name=f"sl{blk}_{hp}_{qc}_{mt}",
                            )
                            nc.tensor.matmul(
                                sl[:, 0:512],
                                kt[0:64, hp, mt * 128 : (mt + 1) * 128],
                                qt_sb[0:64, hp, qc * 512 : (qc + 1) * 512],
                                start=True,
                                stop=True,
                            )
                            nc.tensor.matmul(
                                sl[:, 512:1024],
                                kt[64:128, hp, mt * 128 : (mt + 1) * 128],
                                qt_sb[64:128, hp, qc * 512 : (qc + 1) * 512],
                                start=True,
                                stop=True,
                            )
                            psl = p_pool.tile(
                                [128, 1024], F32R, tag="p",
                                name=f"psl{blk}_{hp}_{qc}_{mt}",
                            )
                            nc.scalar.activation(psl, sl, AF.Exp, scale=0.125)

                            def o_pair(mt=mt, psl=psl, ops_e=ops_e, ops_o=ops_o):
                                nc.tensor.matmul(
                                    ops_e,
                                    vt[:, mt, 2 * hp, :],
                                    psl[:, 0:512],
                                    start=(mt == 0),
                                    stop=(mt == MS - 1),
                                )
                                nc.tensor.matmul(
                                    ops_o,
                                    vt[:, mt, 2 * hp + 1, :],
                                    psl[:, 512:1024],
                                    start=(mt == 0),
                                    stop=(mt == MS - 1),
                                )

                            o_emits.append(o_pair)
                            # software pipeline: O lags S by one slab
                            if mt >= 1:
                                o_emits.pop(0)()
                            # sprinkle next block's projection work
                            for _ in range(per):
                                if pend:
                                    pend.pop(0)()
                            slab_i += 1
                        while o_emits:
                            o_emits.pop(0)()
                        # flush to accumulators
                        je = hp * 4 + 0 * 2 + qc
                        jo = hp * 4 + 1 * 2 + qc
                        if blk == 0:
                            nc.vector.tensor_copy(acc_o[:, je, :], ops_e)
                            nc.vector.tensor_copy(acc_o[:, jo, :], ops_o)
                        else:
                            nc.vector.tensor_add(
                                acc_o[:, je, :], acc_o[:, je, :], ops_e
                            )
                            nc.vector.tensor_add(
                                acc_o[:, jo, :], acc_o[:, jo, :], ops_o
                            )
                for t in pend:  # any leftovers
                    t()
                if next_st is not None:
                    cur_st = next_st

        # ---- P3: normalization + repack ----
        with ExitStack() as nctx:
            norm = nctx.enter_context(tc.tile_pool(name="norm", bufs=1))
            bps = nctx.enter_context(
                tc.tile_pool(name="bps", bufs=4, space="PSUM")
            )
            ops2 = nctx.enter_context(
                tc.tile_pool(name="ops2", bufs=4, space="PSUM")
            )
            out_pool = nctx.enter_context(tc.tile_pool(name="outp", bufs=2))

            # move l rows (partition 64) to 16 partitions via SBUF->SBUF DMA,
            # reciprocal in parallel lanes, then gather back to partition 0
            recip16 = norm.tile([16, 512], F32R, tag="recip16")
            nc.sync.dma_start(out=recip16, in_=acc_o[64:65, :, :].bitcast(F32R))
            with nc.allow_low_precision(reason="1/l in fp32r is fine"):
                nc.vector.reciprocal(recip16[:], recip16[:])
            recip = norm.tile([1, 16, 512], F32R, tag="recip")
            nc.sync.dma_start(out=recip, in_=recip16[:, :])
            for j in range(16):
                bp = bps.tile([64, 512], F32, tag="bps")
                nc.tensor.matmul(
                    bp,
                    ones_row[0:1, 0:64],
                    recip[:, j, :],
                    start=True,
                    stop=True,
                )
                nc.vector.tensor_mul(
                    acc_o[0:64, j, :].bitcast(F32R), acc_o[0:64, j, :], bp
                )

            # repack normalized O^T into pair-packed fp32r lhsT layout:
            # ko_sb[0:64, hp*2+qc, :]   = head 2hp
            # ko_sb[64:128, hp*2+qc, :] = head 2hp+1
            ko_sb = norm.tile([128, H, 512], F32R, tag="ko")
            for hp in range(HP):
                for qc in range(QC):
                    j2 = hp * 2 + qc
                    nc.sync.dma_start(
                        out=ko_sb[0:64, j2, :],
                        in_=acc_o[0:64, hp * 4 + qc, :].bitcast(F32R),
                    )
                    nc.sync.dma_start(
                        out=ko_sb[64:128, j2, :],
                        in_=acc_o[0:64, hp * 4 + 2 + qc, :].bitcast(F32R),
                    )

            # ---- P4: out projection ----
            for qt_i in range(NQ // 128):
                qc = qt_i // 4
                ql = qt_i % 4
                ob = out_pool.tile([128, DQ], F32, tag="outp")
                for nck in range(DQ // 512):
                    pp = ops2.tile([128, 512], F32, tag="ops2")
                    for hp in range(HP):
                        nc.tensor.matmul(
                            pp,
                            ko_sb[:, hp * 2 + qc, ql * 128 : (ql + 1) * 128],
                            wo_sb[:, hp, nck * 512 : (nck + 1) * 512],
                            start=(hp == 0),
                            stop=False,
                        )
                    nc.tensor.matmul(
                        pp,
                        ones_row[0:1, :],
                        bo_sb[0:1, nck * 512 : (nck + 1) * 512],
                        start=False,
                        stop=True,
                    )
                    nc.vector.tensor_copy(ob[:, nck * 512 : (nck + 1) * 512], pp)
                nc.sync.dma_start(
                    out=out[qt_i * 128 : (qt_i + 1) * 128, :], in_=ob
                )


_NC_CACHE = None


def _get_nc():
    global _NC_CACHE
    if _NC_CACHE is None:
        _NC_CACHE = build_nc()
    return _NC_CACHE


def shard_inputs(x, context, Wq, Wk, Wv, Wo, bo):
    ones = np.ones((1, 128), np.float32)
    bo2 = np.ascontiguousarray(np.asarray(bo, np.float32).reshape(1, DQ))
    Wq = np.ascontiguousarray(np.asarray(Wq, np.float32))
    Wk = np.ascontiguousarray(np.asarray(Wk, np.float32))
    Wv = np.ascontiguousarray(np.asarray(Wv, np.float32))
    Wo = np.ascontiguousarray(np.asarray(Wo, np.float32))
    maps = []
    for c in range(NCORES):
        b = c // 4
        r0 = (c % 4) * NQ
        maps.append(
            {
                "xT": np.ascontiguousarray(x[b, r0 : r0 + NQ, :].T),
                "ctxT": np.ascontiguousarray(context[b].T),
                "wq": Wq,
                "wk": Wk,
                "wv": Wv,
                "wo": Wo,
                "bo": bo2,
                "ones_d": ones,
            }
        )
    return maps


def kernel(x, context, Wq, Wk, Wv, Wo, bo):
    from concourse.bass_utils import run_bass_kernel_spmd

    x = np.asarray(x, np.float32)
    context = np.asarray(context, np.float32)
    maps = shard_inputs(x, context, Wq, Wk, Wv, Wo, bo)
    nc = _get_nc()
    trace = os.environ.get("KERNEL_TRACE", "0") == "1"
    res = run_bass_kernel_spmd(
        nc, maps, core_ids=list(range(NCORES)), trace=trace
    )
    full = np.empty((B, N, DQ), np.float32)
    for c in range(NCORES):
        b = c // 4
        r0 = (c % 4) * NQ
        full[b, r0 : r0 + NQ, :] = res.results[c]["out"]
    if trace:
        kernel.last_exec_time_ns = res.exec_time_ns
    return full



# revision 14
# speedup vs baseline: 1.2215x; 1.2215x over previous
"""Cross-attention Trainium2 kernel (nn_CrossAttention_24575802868332).

Sharding: 8 cores; core c handles batch b = c//4 and query rows
r = (c%4)*1024 .. +1024.  Embarrassingly parallel, no collectives.
Host pre-transposes x/context slices and casts to bf16.

v2 design (vs baseline): q-outer two-chunk schedule with K/V resident
in SBUF (bf16), so the normalize + output projection of chunk 0 hides
under chunk 1's attention and only chunk 1's tail is exposed.  All
input DMAs are chunked and interleaved so the PE starts ~1us in.
Attention matmuls are bf16 (FWL weight loads); accumulation fp32.

Per-core schedule:
  P1: q^T = Wq^T x^T (kc-outer, 8 psum banks from the shared pools),
      interleaved with per-kc wq/xt DMA arrivals -> qt bf16 [128,4,1024]
  chunk c in {0,1} (q cols c*512..+512):
    for blk 0..7: 16 slabs (hp x mt):
      S^T pair -> psum [128,1024]; exp (scalar, 0.125 scale) -> bf16
      psl; O_aug pair accumulates into [65,512] psums (ones col of
      v_aug gives the softmax denominator l in row 64); per-hp flush
      to acc_c (fp32 SBUF)
    chunk0 sprinkles next block's K/V projection thunks (K^T via Wk
    lhsT; V via ctx lhsT) into the slab stream; K/V stay resident:
    kt [128,4,4096] bf16, vt [128,32,8,65] bf16.
    chunk1 sprinkles chunk0's P3 (1/l bcast + normalize + pair repack)
    and P4 (out = O/l @ Wo + bo) and output DMAs instead.
  tail: P3+P4 of chunk1 only (~12us exposed).
"""

import os
import sys

sys.path.insert(0, "/opt/trn_rl_repo")

from contextlib import ExitStack

import numpy as np

import concourse.bass as bass
import concourse.tile as tile
from concourse import bacc, mybir

F32 = mybir.dt.float32
F32R = mybir.dt.float32r
BF16 = mybir.dt.bfloat16
AF = mybir.ActivationFunctionType

# Problem constants (hardcoded per contract)
B, N, M = 2, 4096, 4096
DQ, DC, INNER = 1024, 768, 512
H, D = 8, 64
NCORES = 8
NQ = N * B // NCORES  # 1024 query rows per core
QC = 2  # q chunks of 512
QCW = NQ // QC  # 512
MBLK = 512  # m block size
NBLK = M // MBLK  # 8
HP = H // 2  # 4 head pairs
KQ = DQ // 128  # 8 k-chunks for q proj
KC = DC // 128  # 6 k-chunks for k/v proj
MS = MBLK // 128  # 4 m-subtiles per block


def build_nc():
    nc = bacc.Bacc(
        "TRN2",
        target_bir_lowering=False,
        debug=False,
        enable_asserts=False,
        num_devices=NCORES,
    )
    xT = nc.dram_tensor("xT", [DQ, NQ], BF16, kind="ExternalInput").ap()
    ctxT = nc.dram_tensor("ctxT", [DC, M], BF16, kind="ExternalInput").ap()
    wq = nc.dram_tensor("wq", [DQ, INNER], BF16, kind="ExternalInput").ap()
    wk = nc.dram_tensor("wk", [DC, INNER], BF16, kind="ExternalInput").ap()
    wv = nc.dram_tensor("wv", [DC, INNER], BF16, kind="ExternalInput").ap()
    wo = nc.dram_tensor("wo", [INNER, DQ], BF16, kind="ExternalInput").ap()
    bo = nc.dram_tensor("bo", [1, DQ], F32, kind="ExternalInput").ap()
    ones_d = nc.dram_tensor("ones_d", [1, 128], BF16, kind="ExternalInput").ap()
    out = nc.dram_tensor("out", [NQ, DQ], F32, kind="ExternalOutput").ap()

    with tile.TileContext(nc) as tc:
        _emit(nc, tc, xT, ctxT, wq, wk, wv, wo, bo, ones_d, out)
    nc.compile()
    return nc


def _emit(nc, tc, xT, ctxT, wq, wk, wv, wo, bo, ones_d, out):
    with ExitStack() as ctx:
        consts = ctx.enter_context(tc.tile_pool(name="consts", bufs=1))
        persist = ctx.enter_context(tc.tile_pool(name="persist", bufs=1))
        ctx_pool = ctx.enter_context(tc.tile_pool(name="ctx", bufs=2))
        p_pool = ctx.enter_context(tc.tile_pool(name="p", bufs=3))
        out_pool = ctx.enter_context(tc.tile_pool(name="outp", bufs=2))
        norm = ctx.enter_context(tc.tile_pool(name="norm", bufs=1))
        s_ps = ctx.enter_context(tc.tile_pool(name="sps", bufs=2, space="PSUM"))
        o_ps = ctx.enter_context(tc.tile_pool(name="ops", bufs=4, space="PSUM"))

        # ---- interleaved input DMAs: wq/xt chunks first so P1 starts early
        wq_sb = consts.tile([128, KQ, INNER], BF16, tag="wq")
        xt_sb = consts.tile([128, KQ, NQ], BF16, tag="xt")
        wqr = wq.rearrange("(k p) n -> p k n", p=128)
        xTr = xT.rearrange("(k p) q -> p k q", p=128)
        for kc in range(KQ):
            nc.sync.dma_start(out=wq_sb[:, kc, :], in_=wqr[:, kc, :])
            nc.sync.dma_start(out=xt_sb[:, kc, :], in_=xTr[:, kc, :])
        wk_sb = consts.tile([128, KC, INNER], BF16, tag="wk")
        nc.sync.dma_start(out=wk_sb, in_=wk.rearrange("(k p) n -> p k n", p=128))
        wv_sb = consts.tile([128, KC, INNER], BF16, tag="wv")
        nc.sync.dma_start(out=wv_sb, in_=wv.rearrange("(k p) n -> p k n", p=128))
        ctxTr = ctxT.rearrange("(k p) m -> p k m", p=128)
        wo_sb = consts.tile([128, INNER // 128, DQ], BF16, tag="wo")
        nc.sync.dma_start(out=wo_sb, in_=wo.rearrange("(k p) n -> p k n", p=128))
        ones_row = consts.tile([1, 128], BF16, tag="ones_row")
        nc.sync.dma_start(out=ones_row, in_=ones_d)
        # bias broadcast to all 128 partitions
        bias_bc = consts.tile([128, DQ], F32, tag="bias_bc")
        bias_ap = bass.AP(tensor=bo.tensor, offset=0, ap=[[0, 128], [1, DQ]])
        nc.gpsimd.dma_start(out=bias_bc, in_=bias_ap)

        # persistent attention state
        qt_sb = persist.tile([128, HP, NQ], BF16, tag="qt")
        kt_all = persist.tile([128, HP, M], BF16, tag="kt")
        vt_all = persist.tile([128, NBLK * MS, H, 65], BF16, tag="vt")
        acc = [
            persist.tile([65, H, QCW], F32, tag=f"acc{c}", name=f"acc{c}")
            for c in range(QC)
        ]
        # single-buffered (ring) across chunks: chunk1's writers naturally
        # wait for chunk0's last readers, which finish first
        ko = [
            norm.tile([128, HP, QCW], BF16, tag="ko", name=f"ko{c}")
            for c in range(QC)
        ]
        nodd = [
            norm.tile([64, HP, QCW], BF16, tag="nodd", name=f"nodd{c}")
            for c in range(QC)
        ]
        # v_aug ones column (written once; later evacs only touch cols 0:64)
        nc.vector.memset(vt_all[:, :, :, 64:65], 1.0)

        # ---- P1: q^T projection, kc-outer over 8 shared psum banks ----
        qp = [None] * 8  # slot = it*2 + qh
        for kc in range(KQ):
            for it in range(HP):
                for qh in range(2):
                    s = it * 2 + qh
                    if kc == 0:
                        if it < 2 and qh == 0:
                            big = s_ps.tile(
                                [128, 1024], F32, tag="sps", name=f"qp{it}"
                            )
                            qp[it * 2] = big[:, 0:512]
                            qp[it * 2 + 1] = big[:, 512:1024]
                        elif it >= 2:
                            qp[s] = o_ps.tile(
                                [128, 512], F32, tag="ops", name=f"qp{s}"
                            )
                    nc.tensor.matmul(
                        qp[s],
                        wq_sb[:, kc, it * 128 : (it + 1) * 128],
                        xt_sb[:, kc, qh * 512 : (qh + 1) * 512],
                        start=(kc == 0),
                        stop=(kc == KQ - 1),
                    )
        for it in range(HP):
            for qh in range(2):
                nc.vector.tensor_copy(
                    qt_sb[:, it, qh * 512 : (qh + 1) * 512], qp[it * 2 + qh]
                )

        # ---- K/V projection thunks for one m-block ----
        def make_proj_thunks(blk):
            st = {}
            th = []

            def t_dma(blk=blk):
                cx = ctx_pool.tile([128, KC, MBLK], BF16, tag="cx", name=f"cx{blk}")
                m0 = blk * MBLK
                nc.sync.dma_start(out=cx, in_=ctxTr[:, :, m0 : m0 + MBLK])
                st["cx"] = cx

            th.append(t_dma)
            for it in range(HP):
                for kc in range(KC):
                    def t_kmm(it=it, kc=kc, blk=blk):
                        if kc == 0:
                            st[f"kp{it}"] = o_ps.tile(
                                [128, 512], F32, tag="ops", name=f"kp{blk}_{it}"
                            )
                        nc.tensor.matmul(
                            st[f"kp{it}"],
                            wk_sb[:, kc, it * 128 : (it + 1) * 128],
                            st["cx"][:, kc, :],
                            start=(kc == 0),
                            stop=(kc == KC - 1),
                        )
                    th.append(t_kmm)

                def t_kev(it=it, blk=blk):
                    nc.vector.tensor_copy(
                        kt_all[:, it, blk * MBLK : (blk + 1) * MBLK], st[f"kp{it}"]
                    )

                th.append(t_kev)
            for ms in range(MS):
                for kc in range(KC):
                    def t_vmm(ms=ms, kc=kc, blk=blk):
                        if kc == 0:
                            st[f"vp{ms}"] = o_ps.tile(
                                [128, 512], F32, tag="ops", name=f"vp{blk}_{ms}"
                            )
                        nc.tensor.matmul(
                            st[f"vp{ms}"],
                            st["cx"][:, kc, ms * 128 : (ms + 1) * 128],
                            wv_sb[:, kc, :],
                            start=(kc == 0),
                            stop=(kc == KC - 1),
                        )
                    th.append(t_vmm)

                def t_vev(ms=ms, blk=blk):
                    nc.vector.tensor_copy(
                        vt_all[:, blk * MS + ms, :, 0:64],
                        st[f"vp{ms}"][:].rearrange("p (h d) -> p h d", h=H),
                    )

                th.append(t_vev)
            return th

        # ---- P3: normalize chunk accumulator, repack to pair layout ----
        def make_p3_thunks(c):
            st = {}
            th = []

            def t_rd(c=c):
                r8 = norm.tile([H, QCW], F32R, tag="r8", name=f"r8_{c}")
                nc.sync.dma_start(out=r8, in_=acc[c][64:65, :, :].bitcast(F32R))
                st["r8"] = r8

            def t_rc(c=c):
                rb8 = norm.tile([H, QCW], BF16, tag="rb8", name=f"rb8_{c}")
                with nc.allow_low_precision(reason="1/l in bf16 is fine"):
                    nc.vector.reciprocal(rb8[:], st["r8"][:])
                st["rb8"] = rb8

            def t_rb(c=c):
                rr = norm.tile([1, H, QCW], BF16, tag="rr", name=f"rr_{c}")
                nc.sync.dma_start(out=rr, in_=st["rb8"][:, :])
                st["rr"] = rr

            th += [t_rd, t_rc, t_rb]
            for h in range(H):
                def t_bcast(h=h, c=c):
                    bp = o_ps.tile([64, 512], F32, tag="ops", name=f"bp{c}_{h}")
                    nc.tensor.matmul(
                        bp,
                        ones_row[0:1, 0:64],
                        st["rr"][:, h, :],
                        start=True,
                        stop=True,
                    )
                    st[f"bp{h}"] = bp

                def t_mul(h=h, c=c):
                    dst = (
                        ko[c][0:64, h // 2, :]
                        if h % 2 == 0
                        else nodd[c][:, h // 2, :]
                    )
                    nc.vector.tensor_mul(dst, acc[c][0:64, h, :], st[f"bp{h}"])

                th += [t_bcast, t_mul]
            for hp in range(HP):
                def t_rp(hp=hp, c=c):
                    nc.sync.dma_start(
                        out=ko[c][64:128, hp, :], in_=nodd[c][:, hp, :]
                    )
                th.append(t_rp)
            return th

        # ---- P4: out = (O/l) @ Wo + bo for one chunk ----
        def make_p4_thunks(c):
            st = {}
            th = []
            for qi in range(QCW // 128):
                def t_alloc(qi=qi, c=c):
                    st[f"ob{qi}"] = out_pool.tile(
                        [128, DQ], F32, tag="ob", name=f"ob{c}_{qi}"
                    )

                th.append(t_alloc)
                for nck in range(DQ // 512):
                    for hp in range(HP):
                        def t_mm(qi=qi, nck=nck, hp=hp, c=c):
                            if hp == 0:
                                st["pp"] = o_ps.tile(
                                    [128, 512], F32, tag="ops",
                                    name=f"pp{c}_{qi}_{nck}",
                                )
                            nc.tensor.matmul(
                                st["pp"],
                                ko[c][:, hp, qi * 128 : (qi + 1) * 128],
                                wo_sb[:, hp, nck * 512 : (nck + 1) * 512],
                                start=(hp == 0),
                                stop=(hp == HP - 1),
                            )
                        th.append(t_mm)

                    def t_ev(qi=qi, nck=nck):
                        nc.vector.tensor_add(
                            st[f"ob{qi}"][:, nck * 512 : (nck + 1) * 512],
                            st["pp"],
                            bias_bc[:, nck * 512 : (nck + 1) * 512],
                        )

                    th.append(t_ev)

                def t_dma(qi=qi, c=c):
                    r0 = c * QCW + qi * 128
                    nc.sync.dma_start(
                        out=out[r0 : r0 + 128, :], in_=st[f"ob{qi}"]
                    )

                th.append(t_dma)
            return th

        # ---- attention slab stream ----
        def run_chunk(c, pend, spread_all=False):
            """16 slabs per block; pop background thunks evenly.

            spread_all: amortize the initial pend over the whole chunk
            (used for chunk 1's P3/P4 backlog) instead of per block;
            chunk 0 instead appends block b+1's projection thunks at
            block b and drains them within that block.
            """
            q0 = c * QCW
            per_global = (
                (len(pend) + NBLK * HP * MS - 1) // (NBLK * HP * MS)
                if (spread_all and pend)
                else 0
            )
            for blk in range(NBLK):
                if c == 0 and blk + 1 < NBLK:
                    pend = pend + make_proj_thunks(blk + 1)
                n_slabs = HP * MS
                if spread_all:
                    per = per_global
                else:
                    per = (len(pend) + n_slabs - 1) // n_slabs if pend else 0
                for hp in range(HP):
                    ops_e = o_ps.tile(
                        [65, 512], F32, tag="ops", name=f"oe{c}_{blk}_{hp}"
                    )
                    ops_o = o_ps.tile(
                        [65, 512], F32, tag="ops", name=f"oo{c}_{blk}_{hp}"
                    )
                    o_emits = []
                    for mt in range(MS):
                        sl = s_ps.tile(
                            [128, 1024], F32, tag="sps",
                            name=f"sl{c}_{blk}_{hp}_{mt}",
                        )
                        mofs = blk * MBLK + mt * 128
                        nc.tensor.matmul(
                            sl[:, 0:512],
                            kt_all[0:64, hp, mofs : mofs + 128],
                            qt_sb[0:64, hp, q0 : q0 + QCW],
                            start=True,
                            stop=True,
                        )
                        nc.tensor.matmul(
                            sl[:, 512:1024],
                            kt_all[64:128, hp, mofs : mofs + 128],
                            qt_sb[64:128, hp, q0 : q0 + QCW],
                            start=True,
                            stop=True,
                        )
                        psl = p_pool.tile(
                            [128, 1024], BF16, tag="p",
                            name=f"psl{c}_{blk}_{hp}_{mt}",
                        )
                        nc.scalar.activation(psl, sl, AF.Exp, scale=0.125)

                        def o_pair(mt=mt, psl=psl, ops_e=ops_e, ops_o=ops_o,
                                   blk=blk, hp=hp):
                            bms = blk * MS + mt
                            nc.tensor.matmul(
                                ops_e,
                                vt_all[:, bms, 2 * hp, :],
                                psl[:, 0:512],
                                start=(mt == 0),
                                stop=(mt == MS - 1),
                            )
                            nc.tensor.matmul(
                                ops_o,
                                vt_all[:, bms, 2 * hp + 1, :],
                                psl[:, 512:1024],
                                start=(mt == 0),
                                stop=(mt == MS - 1),
                            )

                        o_emits.append(o_pair)
                        if mt >= 1:
                            o_emits.pop(0)()
                        for _ in range(per):
                            if pend:
                                pend.pop(0)()
                    while o_emits:
                        o_emits.pop(0)()
                    # flush O_aug psums into the chunk accumulator
                    if blk == 0:
                        nc.vector.tensor_copy(acc[c][:, 2 * hp, :], ops_e)
                        nc.vector.tensor_copy(acc[c][:, 2 * hp + 1, :], ops_o)
                    else:
                        nc.vector.tensor_add(
                            acc[c][:, 2 * hp, :], acc[c][:, 2 * hp, :], ops_e
                        )
                        nc.vector.tensor_add(
                            acc[c][:, 2 * hp + 1, :], acc[c][:, 2 * hp + 1, :],
                            ops_o,
                        )
            for t in pend:
                t()

        # block 0's projections must fully precede chunk 0's slab
        # stream (the in-order PE queue would deadlock otherwise)
        for t in make_proj_thunks(0):
            t()
        run_chunk(0, [])
        run_chunk(1, make_p3_thunks(0) + make_p4_thunks(0), spread_all=True)
        for t in make_p3_thunks(1) + make_p4_thunks(1):
            t()


_NC_CACHE = None


def _get_nc():
    global _NC_CACHE
    if _NC_CACHE is None:
        _NC_CACHE = build_nc()
    return _NC_CACHE


def shard_inputs(x, context, Wq, Wk, Wv, Wo, bo):
    import ml_dtypes

    bf16 = ml_dtypes.bfloat16
    ones = np.ones((1, 128), np.float32).astype(bf16)
    bo2 = np.ascontiguousarray(np.asarray(bo, np.float32).reshape(1, DQ))
    Wq = np.ascontiguousarray(np.asarray(Wq, np.float32).astype(bf16))
    Wk = np.ascontiguousarray(np.asarray(Wk, np.float32).astype(bf16))
    Wv = np.ascontiguousarray(np.asarray(Wv, np.float32).astype(bf16))
    Wo = np.ascontiguousarray(np.asarray(Wo, np.float32).astype(bf16))
    maps = []
    for c in range(NCORES):
        b = c // 4
        r0 = (c % 4) * NQ
        maps.append(
            {
                "xT": np.ascontiguousarray(x[b, r0 : r0 + NQ, :].T.astype(bf16)),
                "ctxT": np.ascontiguousarray(context[b].T.astype(bf16)),
                "wq": Wq,
                "wk": Wk,
                "wv": Wv,
                "wo": Wo,
                "bo": bo2,
                "ones_d": ones,
            }
        )
    return maps


def kernel(x, context, Wq, Wk, Wv, Wo, bo):
    from concourse.bass_utils import run_bass_kernel_spmd

    x = np.asarray(x, np.float32)
    context = np.asarray(context, np.float32)
    maps = shard_inputs(x, context, Wq, Wk, Wv, Wo, bo)
    nc = _get_nc()
    trace = os.environ.get("KERNEL_TRACE", "0") == "1"
    res = run_bass_kernel_spmd(
        nc, maps, core_ids=list(range(NCORES)), trace=trace
    )
    full = np.empty((B, N, DQ), np.float32)
    for c in range(NCORES):
        b = c // 4
        r0 = (c % 4) * NQ
        full[b, r0 : r0 + NQ, :] = res.results[c]["out"]
    if trace:
        kernel.last_exec_time_ns = res.exec_time_ns
    return full
